# revision 13
# baseline (speedup 1.0000x reference)
"""Dense-MoE FFN kernel for TRN2, expert-parallel over 8 NeuronCores with
sparse token dispatch.

Reference computation (token t, hidden H=1024, ffn F=4096, E=8 experts,
top-K=2 routing):
    y_e = gelu_tanh(x @ w1_e + b1_e) @ w2_e + b2_e     (reference runs dense)
    weight[t, e] = sum_k probs[k, t] * (experts[k, t] == e)
    out[t] = sum_e weight[t, e] * y_e[t]

Tokens with weight[t, e] == 0 contribute exactly 0 to the sum, so each
expert only needs its routed tokens (~T*K/E plus dedupe, ~960 of 4096).

Sharding: expert-parallel. Core c holds expert c's weights. The host routes:
it gathers core c's tokens (x columns + combine weights) into a fixed
capacity-1024 buffer, the device computes
    out_sel[t'] = weight[t'] * (gelu_tanh(x_sel[t'] @ w1 + b1) @ w2 + b2)
and the host scatter-adds the partials into the full output (the unshard /
"all-reduce" step of the masked sum). If routing ever exceeds capacity,
larger variants of the same kernel (1280, then all-4096-tokens) keep the
result correct for any input distribution.

Device kernel (per core):
  - x arrives pre-gathered, pre-transposed (xT_sel [H, CAP]) and fp16-cast,
    so the contraction dim lands on SBUF partitions with no on-device
    transpose. fp16 runs the PE at full rate (4x over fp32) with ~4e-4
    relative error (11-bit significand, fp32 PSUM accumulation).
  - matmul1: h^T[f, t'] += w1[h_k, f_m].T @ xT[h_k, t']
  - gelu (tanh approx, matching jax.nn.gelu) fused with +b1 on ScalarE
  - matmul2: y[t', hh] += h^T[f_k, t'_m].T @ w2[f_k, hh], plus a rank-1
    ones.T @ b2 accumulation for the bias
  - combine: out[t', hh] = wvec[t'] * y[t', hh] on VectorE
"""

import numpy as np

import concourse.mybir as mybir
import concourse.tile as tile
from concourse import bacc
from concourse.bass_utils import run_bass_kernel_spmd

# Problem shapes (hardcoded per contract).
B, S, H, F, E, K = 2, 2048, 1024, 4096, 8, 2
T = B * S  # 4096 tokens

N_CORES = 8
PSA_BUFS = 3
PSB_BUFS = 3
XT_BUFS = 2
OUT_BUFS = 4
CAP_SPARSE = 1024
CHUNKS_SPARSE = (512, 512)
CHUNKS_MID = (512, 512, 256)
CHUNKS_DENSE = (512,) * 8

F_TILES = F // 128  # 32
H_TILES = H // 128  # 8

f16 = mybir.dt.float16
f32 = mybir.dt.float32


def _build_nc(chunks, loop_n: int = 0):
    """Build the per-core Bass module for sum(chunks) gathered tokens.

    loop_n is a benchmarking-only knob (repeat the body in a hardware
    For_i loop); production kernel() uses the default.
    """
    cap = sum(chunks)
    nc = bacc.Bacc(None, target_bir_lowering=False)

    xT_d = nc.dram_tensor("xT", [H, cap], f16, kind="ExternalInput")
    w1_d = nc.dram_tensor("w1", [H, F], f16, kind="ExternalInput")
    w2_d = nc.dram_tensor("w2", [F, H], f16, kind="ExternalInput")
    b1_d = nc.dram_tensor("b1T", [128, F_TILES], f32, kind="ExternalInput")
    b2_d = nc.dram_tensor("b2r", [1, H], f16, kind="ExternalInput")
    wv_d = nc.dram_tensor("wvec", [128, cap // 128], f32, kind="ExternalInput")
    out_d = nc.dram_tensor("out", [cap, H], f32, kind="ExternalOutput")

    with tile.TileContext(nc) as tc:
        with (
            tc.tile_pool(name="const", bufs=1) as constp,
            tc.tile_pool(name="xt", bufs=XT_BUFS) as xtp,
            tc.tile_pool(name="h", bufs=1) as hp,
            tc.tile_pool(name="outsb", bufs=OUT_BUFS) as outp,
            tc.tile_pool(name="psA", bufs=PSA_BUFS, space="PSUM") as psA,
            tc.tile_pool(name="psB", bufs=PSB_BUFS, space="PSUM") as psB,
        ):
            # ---- resident weights / constants ----
            w1_sb = constp.tile([128, H_TILES * F], f16)
            for hk in range(H_TILES):
                nc.sync.dma_start(
                    w1_sb[:, hk * F : (hk + 1) * F],
                    w1_d[hk * 128 : (hk + 1) * 128, :],
                )
            w2_sb = constp.tile([128, F_TILES * H], f16)
            for fk in range(F_TILES):
                nc.sync.dma_start(
                    w2_sb[:, fk * H : (fk + 1) * H],
                    w2_d[fk * 128 : (fk + 1) * 128, :],
                )
            b1_sb = constp.tile([128, F_TILES], f32)
            nc.sync.dma_start(b1_sb[:], b1_d[:])
            b2_sb = constp.tile([1, H], f16)
            nc.sync.dma_start(b2_sb[:], b2_d[:])
            ones_sb = constp.tile([1, 128], f16)
            nc.vector.memset(ones_sb[:], 1.0)
            wvec_sb = constp.tile([128, cap // 128], f32)
            nc.sync.dma_start(wvec_sb[:], wv_d[:])

            def emit_body():
                off = 0
                for tc_sz in chunks:
                    # stream this chunk of gathered xT
                    xt_sb = xtp.tile([128, H_TILES * tc_sz], f16, name="xt_sb")
                    for hk in range(H_TILES):
                        nc.sync.dma_start(
                            xt_sb[:, hk * tc_sz : (hk + 1) * tc_sz],
                            xT_d[hk * 128 : (hk + 1) * 128, off : off + tc_sz],
                        )

                    # phase A: h^T = gelu(w1.T @ xT + b1), fp16
                    h_sb = hp.tile([128, F_TILES * tc_sz], f16, name="h_sb")
                    for fm in range(F_TILES):
                        ps = psA.tile([128, tc_sz], f32, name="psa")
                        for hk in range(H_TILES):
                            nc.tensor.matmul(
                                ps[:],
                                w1_sb[:, hk * F + fm * 128 : hk * F + (fm + 1) * 128],
                                xt_sb[:, hk * tc_sz : (hk + 1) * tc_sz],
                                start=(hk == 0),
                                stop=(hk == H_TILES - 1),
                            )
                        nc.scalar.activation(
                            h_sb[:, fm * tc_sz : (fm + 1) * tc_sz],
                            ps[:],
                            mybir.ActivationFunctionType.Gelu_apprx_tanh,
                            bias=b1_sb[:, fm : fm + 1],
                        )

                    # phase B: y = h^T.T @ w2 (+ ones.T @ b2), scale, store
                    for tm in range(tc_sz // 128):
                        wcol = (off + tm * 128) // 128
                        for nn in range(H // 512):
                            ps = psB.tile([128, 512], f32, name="psb")
                            for fk in range(F_TILES):
                                nc.tensor.matmul(
                                    ps[:],
                                    h_sb[
                                        :,
                                        fk * tc_sz + tm * 128 : fk * tc_sz
                                        + (tm + 1) * 128,
                                    ],
                                    w2_sb[
                                        :, fk * H + nn * 512 : fk * H + (nn + 1) * 512
                                    ],
                                    start=(fk == 0),
                                    stop=False,
                                )
                            nc.tensor.matmul(
                                ps[:],
                                ones_sb[:, :],
                                b2_sb[:, nn * 512 : (nn + 1) * 512],
                                start=False,
                                stop=True,
                            )
                            o_sb = outp.tile([128, 512], f32, name="o_sb")
                            nc.vector.tensor_scalar_mul(
                                o_sb[:], ps[:], wvec_sb[:, wcol : wcol + 1]
                            )
                            nc.sync.dma_start(
                                out_d[
                                    off + tm * 128 : off + (tm + 1) * 128,
                                    nn * 512 : (nn + 1) * 512,
                                ],
                                o_sb[:],
                            )
                    off += tc_sz

            if loop_n:
                import os as _os
                kw = {}
                if _os.environ.get("BENCH_STAGGER"):
                    kw["staggered_reset"] = True
                if _os.environ.get("BENCH_HINT"):
                    kw["hint_engines"] = (mybir.EngineType.PE,)
                with tc.For_i(0, loop_n, 1, **kw):
                    emit_body()
            else:
                emit_body()

    nc.compile()
    return nc


_NC_CACHE = {}


def _get_nc(chunks=CHUNKS_SPARSE):
    if chunks not in _NC_CACHE:
        _NC_CACHE[chunks] = _build_nc(chunks)
    return _NC_CACHE[chunks]


def _route(probs, experts):
    """Per-core routed token indices and combine weights.

    Returns (idx_list, w_list): token indices (unique, sorted) routed to
    each expert and the summed probs for those tokens.
    """
    pf = np.asarray(probs, dtype=np.float32).reshape(K, T)
    ef = np.asarray(experts).reshape(K, T)
    idx_list, w_list = [], []
    for c in range(N_CORES):
        m = ef == c  # [K, T]
        sel = m.any(axis=0)
        idx = np.nonzero(sel)[0]
        w = (pf * m).sum(axis=0)[idx]
        idx_list.append(idx)
        w_list.append(w.astype(np.float32))
    return idx_list, w_list


def _prep_in_maps(x, probs, experts, w1, b1, w2, b2, cap=CAP_SPARSE, route=None):
    x = np.asarray(x, dtype=np.float32).reshape(T, H)
    xT = np.ascontiguousarray(x.T).astype(np.float16)          # [H, T]
    w1f = np.asarray(w1, dtype=np.float32).astype(np.float16)  # [E, H, F]
    w2f = np.asarray(w2, dtype=np.float32).astype(np.float16)  # [E, F, H]
    b1f = np.asarray(b1, dtype=np.float32)                     # [E, F]
    b2f = np.asarray(b2, dtype=np.float32).astype(np.float16)  # [E, H]
    if route is None:
        route = _route(probs, experts)
    idx_list, w_list = route

    in_maps = []
    for c in range(N_CORES):
        idx, w = idx_list[c], w_list[c]
        n = len(idx)
        xsel = np.zeros((H, cap), dtype=np.float16)
        xsel[:, :n] = xT[:, idx]
        wv = np.zeros(cap, dtype=np.float32)
        wv[:n] = w
        in_maps.append(
            {
                "xT": xsel,
                "w1": np.ascontiguousarray(w1f[c]),
                "w2": np.ascontiguousarray(w2f[c]),
                "b1T": np.ascontiguousarray(b1f[c].reshape(F // 128, 128).T),
                "b2r": b2f[c].reshape(1, H),
                "wvec": np.ascontiguousarray(wv.reshape(cap // 128, 128).T),
            }
        )
    return in_maps


def _unshard(results, route):
    idx_list, _ = route
    out = np.zeros((T, H), dtype=np.float32)
    for c in range(N_CORES):
        idx = idx_list[c]
        out[idx] += results[c]["out"][: len(idx)]
    return out.reshape(B, S, H)


def kernel(x, probs, experts, w1, b1, w2, b2):
    route = _route(probs, experts)
    max_n = max(len(i) for i in route[0])
    # capacity ladder: the reference distribution peaks at ~992 routed
    # tokens per expert (mean 960, sigma 27); larger variants exist only as
    # correctness fallbacks for other input distributions.
    if max_n <= CAP_SPARSE:
        chunks = CHUNKS_SPARSE
    elif max_n <= sum(CHUNKS_MID):
        chunks = CHUNKS_MID
    else:
        chunks = CHUNKS_DENSE
    nc = _get_nc(chunks)
    in_maps = _prep_in_maps(
        x, probs, experts, w1, b1, w2, b2, cap=sum(chunks), route=route
    )
    res = run_bass_kernel_spmd(nc, in_maps, core_ids=list(range(N_CORES)))
    return _unshard(res.results, route)
